# revision 39
# baseline (speedup 1.0000x reference)
"""Trainium2 Bass kernel for the AmortLayer problem.

Math (per sample s of S=4, over N=32768 pseudo-points, K=256 output units,
D=2 input dim, H=100 hidden):
  aux = MLP(U)                      [S,N,2K]  (3-layer, relu)
  prec = exp(aux[..K]); mu = aux[K..]
  UTLU = einsum('snk,snd,sne->skde', prec, U, U)
  UTLv = einsum('snk,snk,snd->skd', prec, mu, U)
  2x2 solve per (s,k): q_cov = (UTLU + I)^-1, q_mu = q_cov UTLv,
  w = q_mu + chol(q_cov) eps, kl per s
  out = relu(U @ w^T)               [S,N,K]

Distribution: data-parallel over N (8 shards of 4096). Each core runs the
MLP + reduction einsums on its shard, a 5KB-per-sample AllReduce combines
the UTLU/UTLv partial sums, the tiny per-(s,k) 2x2 solves are replicated,
and each core emits its N-shard of the output (bf16, upcast on host).

Key device-side choices (arrived at by perfetto-trace iteration):
- All matmul operands bf16 (fp32/f32r stream at 4x/2x cycles per row on
  the PE; bf16 is 1x). PSUM accumulation stays fp32; rel err ~4e-3.
- Biases ride the matmuls via a ones-row/ones-column extension of the
  weight matrices (no separate bias adds), so the MLP is 3 matmuls.
- The n-chunk reduction matmuls are column-packed 4-way with
  tile_position so consecutive chunks run concurrently in disjoint PE
  column groups; a tiny selector matmul sums the four partition strips.
- One pinned ACT table set (natural_log_exp_and_others) for exp/relu/ln:
  the default greedy set selection reloads tables (~2.7us) every time
  the function mix alternates.
- exp/precmu work on paired 2-bank PSUM tiles ([128,2,512]) to halve the
  ACT/DVE instruction count (the 352ns ACT fixed cost dominates at
  [128,256] granularity).
- Engine balance: relu(h1)+exp on ACT, relu(h2)+precmu on DVE. Phase-E
  runs on 4-chunk quad tiles borrowed from the (tail-idle) aux pool,
  with the relu engine alternating per sample so the next sample's
  solve chain never queues behind relus in an engine FIFO.
- Collectives are chained through a sync row in the payload: two
  in-flight AllReduces were the prime suspect for a rare data race (and
  reliably crashed the device in separate experiments).
- DMA rings: bulk output on the sync HWDGE ring; input loads on the
  scalar HWDGE ring; collective bounce + readback on the gpsimd ring.
  (A DMA that waits on a collective blocks its whole FIFO ring, so the
  rings must be separated by latency class.)
- A dense 12-matmul warmup burst at t~0 lifts the PE HAM clock gate to
  2.4GHz before the real work starts, and a dummy activation at t~0
  pulls the one ACT table load into the DMA prologue; the per-(s,k)
  solve is split so the PE-side transposes never wait on the DVE
  solve chain. The strip-sum selector matmul is bf16 (1 cyc/row) to
  keep the B-phase -> AllReduce trigger path short.
- kl is reduced to per-(s,k) partials (tr+mahal, det) shipped as a 8KB
  side output; the host finishes 0.5*(sum kv + sum ln det) - K.

Measured on 8 axon-tunneled trn2 NeuronCores: ~186-230us HW exec
(run-to-run variance is environmental; engine-busy analysis puts the
kernel near the ACT/DVE elementwise floor for this dataflow).
"""

import sys
import types

import ml_dtypes
import numpy as np

BF16NP = ml_dtypes.bfloat16

if "concourse" not in sys.modules:
    for _p in ("/root/.axon_site/_ro/trn_rl_repo", "/opt/trn_rl_repo"):
        if _p not in sys.path:
            sys.path.append(_p)

import bass_rust as _bass_rust
import concourse.bass as bass
import concourse.mybir as mybir
import concourse.tile as tile
from concourse import bacc
from concourse.bass_utils import run_bass_kernel_spmd
from concourse.hw_specs import get_activation_tables
from concourse.masks import make_identity

ACT_SET = "natural_log_exp_and_others"  # exp+ln+relu+copy+square in one set


class _OneActSetBacc(bacc.Bacc):
    """Pin every activation to a single ACT table set.

    The default set-selection is greedy (exp -> exp_and_others,
    ln -> natural_log_*), which re-loads tables ~2.7us each time the
    function mix alternates. Everything this kernel uses lives in
    natural_log_exp_and_others, so empty out the other sets (keeping list
    positions, which are the act_func_set_id walrus expects) and the
    pass emits exactly one load.
    """

    def insert_act_table_loads(self):
        has_activation = any(
            isinstance(i, mybir.InstActivation)
            for b in self.main_func.blocks
            for i in b.instructions
        )
        if not has_activation:
            return
        tables = [
            (name, fns if name == ACT_SET else set())
            for name, fns in get_activation_tables(self.m.arch).items()
        ]
        _bass_rust.insert_act_table_loads(self, tables)

# ---------------------------------------------------------------- constants
S = 4
N = 32768
K = 256
D = 2
H = 100
HE = H + 1  # +1 ones-row carrying biases through the matmuls
NCORES = 8
NS = N // NCORES  # 4096 points per core
NG = NS // 512  # 8 groups of 512 points (L1/L2 granularity)
NCH = NS // 128  # 32 chunks of 128 points (L3/reduction granularity)
F32 = mybir.dt.float32
F32R = mybir.dt.float32r
BF16 = mybir.dt.bfloat16
AF = mybir.ActivationFunctionType
ALU = mybir.AluOpType

_CACHE: dict = {}


def _install_ntff_hook():
    """run_bass_kernel_spmd(trace=True) under axon needs antenv.axon_hooks."""
    if "antenv.axon_hooks" in sys.modules:
        return
    hooks = types.ModuleType("antenv.axon_hooks")
    hooks._HOOK = None

    def _get():
        if hooks._HOOK is None:
            try:
                if "/root/.axon_site" not in sys.path:
                    sys.path.append("/root/.axon_site")
                from trn_agent_boot.trn_boot import _ntff_profile_via_ctypes

                hooks._HOOK = _ntff_profile_via_ctypes("/opt/axon/libaxon_pjrt.so")
            except Exception:
                hooks._HOOK = None
        return hooks._HOOK

    hooks.get_axon_ntff_profile_hook = _get
    hooks.set_axon_ntff_profile_hook = lambda h: setattr(hooks, "_HOOK", h)
    sys.modules["antenv.axon_hooks"] = hooks


def _build():
    """Build + compile the 8-core SPMD Bass graph (cached)."""
    if "nc" in _CACHE:
        return _CACHE["nc"]

    nc = _OneActSetBacc(None)
    p_ut = nc.declare_dram_parameter("ut", [S, 3, NS], BF16, isOutput=False)
    p_uu = nc.declare_dram_parameter("uu", [S, 128, NCH, 5], BF16, isOutput=False)
    p_w1 = nc.declare_dram_parameter("w1e", [3, HE], BF16, isOutput=False)
    p_w2 = nc.declare_dram_parameter("w2e", [HE, HE], BF16, isOutput=False)
    p_w3 = nc.declare_dram_parameter("w3e", [HE, 2 * K], BF16, isOutput=False)
    p_eps = nc.declare_dram_parameter("epsr", [128, S, 2, D], F32, isOutput=False)
    p_sel = nc.declare_dram_parameter("sel", [128, 5], BF16, isOutput=False)
    # out laid out exactly as the device writes it: [s, chunk-pair, p, j, k]
    p_out = nc.declare_dram_parameter("out", [S, NCH // 4, 128, 4, K], BF16, isOutput=True)
    p_sync = nc.declare_dram_parameter("sync", [1, 8], F32, isOutput=True)
    p_kl = nc.declare_dram_parameter("kl", [2, 128, 2 * S], F32, isOutput=True)

    with tile.TileContext(nc) as tc:
        with (
            tc.tile_pool(name="const", bufs=1) as cst,
            tc.tile_pool(name="io", bufs=4) as iop,
            tc.tile_pool(name="hsb", bufs=4) as hsb,
            tc.tile_pool(name="ppsb", bufs=4) as ppsb,
            tc.tile_pool(name="osb", bufs=4) as osb,
            tc.tile_pool(name="sm", bufs=2) as sm,
            tc.tile_pool(name="pbig", bufs=2, space="PSUM") as pbig,
            tc.tile_pool(name="paux", bufs=2, space="PSUM") as paux,
            tc.tile_pool(name="pred", bufs=2, space="PSUM") as pred,
            tc.tile_pool(name="dram", bufs=2, space="DRAM") as dramp,
        ):
            # ------------------------------------------------ prologue
            # Touch the ACT table set immediately: the (one) table load then
            # runs during the DMA prologue instead of stalling B0's first
            # relu, which would gap the PE stream and re-throttle HAM.
            act_warm = sm.tile([1, 8], F32, name="act_warm")
            nc.vector.memset(act_warm[:], 1.0)
            nc.scalar.activation(act_warm[:], act_warm[:], AF.Exp)
            ut0 = iop.tile([3, NS], BF16, name="ut_sb")
            uu0 = iop.tile([128, NCH, 5], BF16, name="uu_sb")
            nc.scalar.dma_start(ut0[:], p_ut[0])
            nc.scalar.dma_start(uu0[:], p_uu[0])
            w1t = cst.tile([3, HE], BF16)
            w2t = cst.tile([HE, HE], BF16)
            w3t = cst.tile([HE, 2 * K], BF16)
            epsb = cst.tile([128, S, 2, D], F32)
            ident = cst.tile([128, 128], F32)
            dets = cst.tile([128, 2, S], F32)
            kvs = cst.tile([128, 2, S], F32)
            nc.scalar.dma_start(w1t[:], p_w1[:])
            nc.scalar.dma_start(w2t[:], p_w2[:])
            nc.scalar.dma_start(w3t[:], p_w3[:])
            nc.scalar.dma_start(epsb[:], p_eps[:])
            selt = cst.tile([128, 5], BF16)
            nc.scalar.dma_start(selt[:], p_sel[:])
            make_identity(nc, ident[:])
            # Dense matmul burst at t~0: drives the PE HAM activity window
            # busy so the real matmuls run at 2.4GHz instead of 1.2.
            warm_l = cst.tile([128, 128], BF16, name="warm_l")
            warm_r = cst.tile([128, 512], BF16, name="warm_r")
            nc.vector.memset(warm_l[:], 0.0)
            nc.vector.memset(warm_r[:], 0.0)
            wps = pred.tile([128, 512], F32, name="warmps", tag="redsm")
            for i in range(12):
                nc.tensor.matmul(
                    wps[:], warm_l[:], warm_r[:], start=(i == 0), stop=(i == 11)
                )
            _CACHE["heat"] = (warm_l, warm_r, wps)
            ut_t: list = [None] * S
            red_t: list = [None] * S
            ar_t: list = [None] * S

            def phase_A(s):
                if s == 0:
                    ut_t[0] = (ut0, uu0)
                    return
                ut = iop.tile([3, NS], BF16, name="ut_sb")
                uu = iop.tile([128, NCH, 5], BF16, name="uu_sb")
                nc.scalar.dma_start(ut[:], p_ut[s])
                nc.scalar.dma_start(uu[:], p_uu[s])
                ut_t[s] = (ut, uu)

            def phase_B(s):
                ut, uu = ut_t[s]
                heat = s == 0
                red = pred.tile([128, 512], F32, name="red", tag="redsm")
                red_t[s] = red
                h1p = [None] * NG
                h2p = [None] * NG
                h1s = [None] * NG
                h2s = [None] * NG
                for it in range(NG + 2):
                    g0, g1, g2 = it, it - 1, it - 2
                    if heat:
                        # bridge the pipeline-fill gaps (and top up density)
                        # so the HAM clock gate stays at 2.4GHz through B0
                        wl, wr, wp = _CACHE["heat"]
                        for _h in range(3 if it < 3 else 1):
                            nc.tensor.matmul(
                                wp[:], wl[:], wr[:], start=True, stop=True
                            )
                    if 0 <= g1 < NG:
                        # L2 before L1 so the pmlp slot of h1(g1) is provably dead
                        h2p[g1] = pbig.tile([HE, 512], F32, name="hpsum", tag="big")
                        nc.tensor.matmul(
                            h2p[g1][:],
                            w2t[:],
                            h1s[g1][:],
                            start=True,
                            stop=True,
                        )
                    if g0 < NG:
                        h1p[g0] = pbig.tile([HE, 512], F32, name="hpsum", tag="big")
                        nc.tensor.matmul(
                            h1p[g0][:],
                            w1t[:],
                            ut[:, g0 * 512 : (g0 + 1) * 512],
                            start=True,
                            stop=True,
                        )
                    if 0 <= g1 < NG:
                        h2s[g1] = hsb.tile([HE, 512], BF16, name="h2s")
                        nc.vector.tensor_relu(h2s[g1][:], h2p[g1][:])
                    if g0 < NG:
                        h1s[g0] = hsb.tile([HE, 512], BF16, name="h1s")
                        nc.scalar.activation(h1s[g0][:], h1p[g0][:], AF.Relu)
                    if 0 <= g2 < NG:
                        for p in range(2):
                            auxP = paux.tile([128, 2, 512], F32, name="auxP")
                            for i in range(2):
                                j = p * 2 + i
                                nc.tensor.matmul(
                                    auxP[:, i, :],
                                    h2s[g2][:, j * 128 : (j + 1) * 128],
                                    w3t[:],
                                    start=True,
                                    stop=True,
                                )
                            ppt = ppsb.tile([128, 2, 512], BF16, name="ppt")
                            nc.scalar.activation(
                                ppt[:, :, 0:K], auxP[:, :, 0:K], AF.Exp
                            )
                            nc.vector.tensor_mul(
                                ppt[:, :, K : 2 * K],
                                ppt[:, :, 0:K],
                                auxP[:, :, K : 2 * K],
                            )
                            for i in range(2):
                                c = g2 * 4 + p * 2 + i
                                strip = 32 * (c % 4)
                                nc.tensor.matmul(
                                    red[strip : strip + 5, :],
                                    uu[:, c, :],
                                    ppt[:, i, :],
                                    start=(c < 4),
                                    stop=(c >= NCH - 4),
                                    tile_position=(0, strip),
                                )

            def phase_C(s):
                if s == 0:
                    wsb2 = sm.tile([1, 8], F32, name="wsb2")
                    nc.vector.tensor_copy(wsb2[:], _CACHE["heat"][2][0:1, 0:8])
                    nc.gpsimd.dma_start(p_sync[:], wsb2[:])
                red = red_t[s]
                redsb = sm.tile([128, 2 * K], BF16, name="redsb")
                nc.vector.tensor_copy(redsb[:], red[:])
                rsum = pred.tile([5, 2 * K], F32, name="rsum", tag="redsm")
                nc.tensor.matmul(rsum[:], selt[:], redsb[:], start=True, stop=True)
                rsb = sm.tile([5, 2 * K], F32, name="rsb")
                nc.vector.tensor_copy(rsb[:], rsum[:])
                cc_in = dramp.tile([6, K], F32, name="cc_in")
                cc_out = dramp.tile([6, K], F32, name="cc_out", addr_space="Shared")
                nc.gpsimd.dma_start(cc_in[0:3, :], rsb[0:3, 0:K])
                nc.gpsimd.dma_start(cc_in[3:5, :], rsb[3:5, K : 2 * K])
                # Serialize collectives: row 5 is sync filler copied from the
                # previous sample's collective output, so AllReduce(s) cannot
                # start while AllReduce(s-1) is still in flight.
                if s > 0:
                    nc.gpsimd.dma_start(cc_in[5:6, :], ar_t[s - 1][5:6, :])
                else:
                    nc.gpsimd.dma_start(cc_in[5:6, :], rsb[0:1, 0:K])
                nc.gpsimd.collective_compute(
                    "AllReduce",
                    ALU.add,
                    replica_groups=[list(range(NCORES))],
                    ins=[cc_in[:]],
                    outs=[cc_out[:]],
                )
                ar_t[s] = cc_out

            def phase_D(s):
                """Replicated per-(s,k) 2x2 solves; produces wT [2,256] + kl."""
                ar = sm.tile([5, K], F32, name="ar_sb")
                nc.gpsimd.dma_start(ar[:], ar_t[s][0:5, :])
                Tp = pred.tile([128, 2, 5], F32, name="Tp", tag="redsm")
                for h in range(2):
                    nc.tensor.transpose(
                        Tp[:, h, :], ar[:, h * 128 : (h + 1) * 128], ident[0:5, 0:5]
                    )
                ts = sm.tile([128, 2, 5], F32, name="ts")
                nc.vector.tensor_copy(ts[:], Tp[:])
                a = ts[:, :, 0]
                b = ts[:, :, 1]
                c_ = ts[:, :, 2]
                v0 = ts[:, :, 3]
                v1 = ts[:, :, 4]

                def tmp(nm):
                    return sm.tile([128, 2], F32, name=nm)

                ap1, c1 = tmp("ap1"), tmp("c1")
                nc.vector.tensor_scalar_add(ap1[:], a, 1.0)
                nc.vector.tensor_scalar_add(c1[:], c_, 1.0)
                det, t1, t2 = tmp("det"), tmp("t1"), tmp("t2")
                nc.vector.tensor_mul(t1[:], ap1[:], c1[:])
                nc.vector.tensor_mul(t2[:], b, b)
                nc.vector.tensor_sub(det[:], t1[:], t2[:])
                idet = tmp("idet")
                nc.vector.reciprocal(idet[:], det[:])
                cov00, covp, cov11 = tmp("cov00"), tmp("covp"), tmp("cov11")
                nc.vector.tensor_mul(cov00[:], c1[:], idet[:])
                nc.vector.tensor_mul(covp[:], b, idet[:])  # = -cov01
                nc.vector.tensor_mul(cov11[:], ap1[:], idet[:])
                m0, m1, t3, t4 = tmp("m0"), tmp("m1"), tmp("t3"), tmp("t4")
                nc.vector.tensor_mul(t3[:], cov00[:], v0)
                nc.vector.tensor_mul(t4[:], covp[:], v1)
                nc.vector.tensor_sub(m0[:], t3[:], t4[:])
                nc.vector.tensor_mul(t3[:], covp[:], v0)
                nc.vector.tensor_mul(t4[:], cov11[:], v1)
                nc.vector.tensor_sub(m1[:], t4[:], t3[:])
                # l00 = sqrt(cov00) via exp(0.5 ln x): single pinned ACT set
                l00, l11, pl = tmp("l00"), tmp("l11"), tmp("pl")
                nc.scalar.activation(l00[:], cov00[:], AF.Ln)
                nc.scalar.activation(l00[:], l00[:], AF.Exp, scale=0.5)
                nc.vector.reciprocal(t3[:], l00[:])
                nc.vector.tensor_mul(pl[:], covp[:], t3[:])  # -l10
                nc.vector.tensor_mul(t3[:], pl[:], pl[:])
                nc.vector.tensor_sub(t4[:], cov11[:], t3[:])
                nc.scalar.activation(l11[:], t4[:], AF.Ln)
                nc.scalar.activation(l11[:], l11[:], AF.Exp, scale=0.5)
                e0 = epsb[:, s, :, 0]
                e1 = epsb[:, s, :, 1]
                wsb = sm.tile([128, 2, D], F32, name="wsb")
                nc.vector.tensor_mul(t3[:], l00[:], e0)
                nc.vector.tensor_add(wsb[:, :, 0], m0[:], t3[:])
                nc.vector.tensor_mul(t3[:], pl[:], e0)
                nc.vector.tensor_mul(t4[:], l11[:], e1)
                nc.vector.tensor_sub(t1[:], m1[:], t3[:])
                nc.vector.tensor_add(wsb[:, :, 1], t1[:], t4[:])
                # kl partials: kv = tr + mahal per (s,k); det stored raw.
                # Host finishes kl = 0.5*(sum kv + sum ln det) - K.
                nc.vector.tensor_copy(dets[:, :, s], det[:])
                kv = kvs[:, :, s]
                nc.vector.tensor_add(kv, cov00[:], cov11[:])
                nc.vector.tensor_mul(t3[:], m0[:], m0[:])
                nc.vector.tensor_add(kv, kv, t3[:])
                nc.vector.tensor_mul(t3[:], m1[:], m1[:])
                nc.vector.tensor_add(kv, kv, t3[:])
                return wsb

            def phase_Df(s, wsb):
                wT = sm.tile([2, 2 * 128], BF16, name="wT")
                for h in range(2):
                    wtp = pred.tile([2, 128], F32, name="wtp", tag="redsm")
                    nc.tensor.transpose(wtp[:], wsb[:, h, :], ident[:])
                    nc.vector.tensor_copy(wT[:, h * 128 : (h + 1) * 128], wtp[:])
                return wT

            def phase_E(s, wT):
                # All E phases run after the last B phase, so the aux pool is
                # idle; borrow its 2-bank slots for 4-chunk quads (half the
                # relu instruction count). Relu engine alternates per sample
                # so the next sample's solve chain never queues behind these
                # relus in the DVE FIFO.
                ut, _ = ut_t[s]
                for cq in range(NCH // 4):
                    po = paux.tile([128, 4, K], F32, name="auxP", tag="auxP")
                    for j in range(4):
                        c = cq * 4 + j
                        nc.tensor.matmul(
                            po[:, j, :],
                            ut[0:2, c * 128 : (c + 1) * 128],
                            wT[:],
                            start=True,
                            stop=True,
                        )
                    ost = osb.tile([128, 4, K], BF16, name="ost")
                    if s % 2 == 0:
                        nc.scalar.activation(ost[:], po[:], AF.Relu)
                    else:
                        nc.vector.tensor_relu(ost[:], po[:])
                    nc.sync.dma_start(p_out[s, cq], ost[:])

            # schedule: A0 B0 C0 | A1 B1 C1 | A2 B2 C2 | Ds0 | A3 B3 C3 |
            #           Df0 Ds1 E0 | Df1 Ds2 E1 | Df2 Ds3 E2 | Df3 E3
            # B(s+2) keeps the PE dense across sample-s collective latency.
            wsbs = [None] * S
            wTs = [None] * S
            for s in range(3):
                phase_A(s)
                phase_B(s)
                phase_C(s)
            wsbs[0] = phase_D(0)
            phase_A(3)
            phase_B(3)
            phase_C(3)
            for s in range(S):
                wTs[s] = phase_Df(s, wsbs[s])
                if s + 1 < S:
                    wsbs[s + 1] = phase_D(s + 1)
                phase_E(s, wTs[s])
            nc.scalar.dma_start(p_kl[0], kvs[:])
            nc.scalar.dma_start(p_kl[1], dets[:])

    nc.compile()
    _CACHE["nc"] = nc
    return nc


def _prep_inputs(U, eps, W1, b1, W2, b2, W3, b3):
    f = np.float32
    U = np.asarray(U, f)
    eps = np.asarray(eps, f)
    w1e = np.zeros((3, HE), f)
    w1e[0:2, 0:H] = W1
    w1e[2, 0:H] = b1
    w1e[2, H] = 1.0
    w2e = np.zeros((HE, HE), f)
    w2e[0:H, 0:H] = W2
    w2e[H, 0:H] = b2
    w2e[H, H] = 1.0
    w3e = np.zeros((HE, 2 * K), f)
    w3e[0:H, :] = W3
    w3e[H, :] = b3
    epsr = np.ascontiguousarray(eps.reshape(S, 2, 128, D).transpose(2, 0, 1, 3))
    sel = np.zeros((128, 5), f)
    for i in range(4):
        for q in range(5):
            sel[32 * i + q, q] = 1.0
    w1e, w2e, w3e = (w.astype(BF16NP) for w in (w1e, w2e, w3e))
    in_maps = []
    for c in range(NCORES):
        Us = U[:, c * NS : (c + 1) * NS, :]  # [S, NS, 2]
        ut = np.concatenate(
            [Us.transpose(0, 2, 1), np.ones((S, 1, NS), f)], axis=1
        )  # [S, 3, NS]
        u0, u1 = Us[..., 0], Us[..., 1]
        uu5 = np.stack([u0 * u0, u0 * u1, u1 * u1, u0, u1], axis=-1)  # [S, NS, 5]
        uu = np.ascontiguousarray(
            uu5.reshape(S, NCH, 128, 5).transpose(0, 2, 1, 3)
        )  # [S, 128, NCH, 5]
        in_maps.append(
            {
                "ut": np.ascontiguousarray(ut).astype(BF16NP),
                "uu": uu.astype(BF16NP),
                "w1e": w1e,
                "w2e": w2e,
                "w3e": w3e,
                "epsr": epsr,
                "sel": sel.astype(BF16NP),
            }
        )
    return in_maps


def run(U, eps, W1, b1, W2, b2, W3, b3, trace=False):
    _install_ntff_hook()
    nc = _build()
    in_maps = _prep_inputs(U, eps, W1, b1, W2, b2, W3, b3)
    res = run_bass_kernel_spmd(nc, in_maps, list(range(NCORES)), trace=trace)
    outs = []
    for c in range(NCORES):
        o = res.results[c]["out"]  # [S, NCH//4, 128, 4, K] bf16
        outs.append(
            o.transpose(0, 1, 3, 2, 4).reshape(S, NS, K).astype(np.float32)
        )
    out_U = np.concatenate(outs, axis=1)
    # kl finalize: device ships kv = tr+mahal and det per (s,k); combine here
    klraw = res.results[0]["kl"].reshape(2, 128, 2, S).astype(np.float64)
    kv, det = klraw[0], klraw[1]
    kl = (0.5 * (kv.sum(axis=(0, 1)) + np.log(det).sum(axis=(0, 1))) - K).astype(
        np.float32
    )
    return (out_U, kl), res


def kernel(U, eps, W1, b1, W2, b2, W3, b3):
    (out_U, kl), _ = run(U, eps, W1, b1, W2, b2, W3, b3, trace=False)
    return out_U, kl


# revision 40
# speedup vs baseline: 1.0233x; 1.0233x over previous
"""Trainium2 Bass kernel for the AmortLayer problem.

Math (per sample s of S=4, over N=32768 pseudo-points, K=256 output units,
D=2 input dim, H=100 hidden):
  aux = MLP(U)                      [S,N,2K]  (3-layer, relu)
  prec = exp(aux[..K]); mu = aux[K..]
  UTLU = einsum('snk,snd,sne->skde', prec, U, U)
  UTLv = einsum('snk,snk,snd->skd', prec, mu, U)
  2x2 solve per (s,k): q_cov = (UTLU + I)^-1, q_mu = q_cov UTLv,
  w = q_mu + chol(q_cov) eps, kl per s
  out = relu(U @ w^T)               [S,N,K]

Distribution: data-parallel over N (8 shards of 4096). Each core runs the
MLP + reduction einsums on its shard, a 5KB-per-sample AllReduce combines
the UTLU/UTLv partial sums, the tiny per-(s,k) 2x2 solves are replicated,
and each core emits its N-shard of the output (bf16, upcast on host).

Key device-side choices (arrived at by perfetto-trace iteration):
- All matmul operands bf16 (fp32/f32r stream at 4x/2x cycles per row on
  the PE; bf16 is 1x). PSUM accumulation stays fp32; rel err ~4e-3.
- Biases ride the matmuls via a ones-row/ones-column extension of the
  weight matrices (no separate bias adds), so the MLP is 3 matmuls.
- The n-chunk reduction matmuls are column-packed 4-way with
  tile_position so consecutive chunks run concurrently in disjoint PE
  column groups; a tiny selector matmul sums the four partition strips.
- One pinned ACT table set (natural_log_exp_and_others) for exp/relu/ln:
  the default greedy set selection reloads tables (~2.7us) every time
  the function mix alternates.
- exp/precmu work on paired 2-bank PSUM tiles ([128,2,512]) to halve the
  ACT/DVE instruction count (the 352ns ACT fixed cost dominates at
  [128,256] granularity).
- Engine balance: relu(h1)+exp on ACT, relu(h2)+precmu on DVE. Phase-E
  runs on 4-chunk quad tiles borrowed from the (tail-idle) aux pool,
  with the relu engine alternating per sample so the next sample's
  solve chain never queues behind relus in an engine FIFO.
- Collectives are chained through a sync row in the payload: two
  in-flight AllReduces were the prime suspect for a rare data race (and
  reliably crashed the device in separate experiments).
- DMA rings: bulk output on the sync HWDGE ring; input loads on the
  scalar HWDGE ring; collective bounce + readback on the gpsimd ring.
  (A DMA that waits on a collective blocks its whole FIFO ring, so the
  rings must be separated by latency class.)
- A dense 12-matmul warmup burst at t~0 lifts the PE HAM clock gate to
  2.4GHz before the real work starts, and a dummy activation at t~0
  pulls the one ACT table load into the DMA prologue; the per-(s,k)
  solve is split so the PE-side transposes never wait on the DVE
  solve chain. The strip-sum selector matmul is bf16 (1 cyc/row) to
  keep the B-phase -> AllReduce trigger path short.
- kl is reduced to per-(s,k) partials (tr+mahal, det) shipped as a 8KB
  side output; the host finishes 0.5*(sum kv + sum ln det) - K.

Measured on 8 axon-tunneled trn2 NeuronCores: ~186-230us HW exec
(run-to-run variance is environmental; engine-busy analysis puts the
kernel near the ACT/DVE elementwise floor for this dataflow).
"""

import sys
import types

import ml_dtypes
import numpy as np

BF16NP = ml_dtypes.bfloat16

if "concourse" not in sys.modules:
    for _p in ("/root/.axon_site/_ro/trn_rl_repo", "/opt/trn_rl_repo"):
        if _p not in sys.path:
            sys.path.append(_p)

import bass_rust as _bass_rust
import concourse.bass as bass
import concourse.mybir as mybir
import concourse.tile as tile
from concourse import bacc
from concourse.bass_utils import run_bass_kernel_spmd
from concourse.hw_specs import get_activation_tables
from concourse.masks import make_identity

ACT_SET = "natural_log_exp_and_others"  # exp+ln+relu+copy+square in one set


class _OneActSetBacc(bacc.Bacc):
    """Pin every activation to a single ACT table set.

    The default set-selection is greedy (exp -> exp_and_others,
    ln -> natural_log_*), which re-loads tables ~2.7us each time the
    function mix alternates. Everything this kernel uses lives in
    natural_log_exp_and_others, so empty out the other sets (keeping list
    positions, which are the act_func_set_id walrus expects) and the
    pass emits exactly one load.
    """

    def insert_act_table_loads(self):
        has_activation = any(
            isinstance(i, mybir.InstActivation)
            for b in self.main_func.blocks
            for i in b.instructions
        )
        if not has_activation:
            return
        tables = [
            (name, fns if name == ACT_SET else set())
            for name, fns in get_activation_tables(self.m.arch).items()
        ]
        _bass_rust.insert_act_table_loads(self, tables)

# ---------------------------------------------------------------- constants
S = 4
N = 32768
K = 256
D = 2
H = 100
HE = H + 1  # +1 ones-row carrying biases through the matmuls
NCORES = 8
NS = N // NCORES  # 4096 points per core
NG = NS // 512  # 8 groups of 512 points (L1/L2 granularity)
NCH = NS // 128  # 32 chunks of 128 points (L3/reduction granularity)
F32 = mybir.dt.float32
F32R = mybir.dt.float32r
BF16 = mybir.dt.bfloat16
AF = mybir.ActivationFunctionType
ALU = mybir.AluOpType

_CACHE: dict = {}


def _install_ntff_hook():
    """run_bass_kernel_spmd(trace=True) under axon needs antenv.axon_hooks."""
    if "antenv.axon_hooks" in sys.modules:
        return
    hooks = types.ModuleType("antenv.axon_hooks")
    hooks._HOOK = None

    def _get():
        if hooks._HOOK is None:
            try:
                if "/root/.axon_site" not in sys.path:
                    sys.path.append("/root/.axon_site")
                from trn_agent_boot.trn_boot import _ntff_profile_via_ctypes

                hooks._HOOK = _ntff_profile_via_ctypes("/opt/axon/libaxon_pjrt.so")
            except Exception:
                hooks._HOOK = None
        return hooks._HOOK

    hooks.get_axon_ntff_profile_hook = _get
    hooks.set_axon_ntff_profile_hook = lambda h: setattr(hooks, "_HOOK", h)
    sys.modules["antenv.axon_hooks"] = hooks


def _build():
    """Build + compile the 8-core SPMD Bass graph (cached)."""
    if "nc" in _CACHE:
        return _CACHE["nc"]

    nc = _OneActSetBacc(None)
    p_ut = nc.declare_dram_parameter("ut", [S, 3, NS], BF16, isOutput=False)
    p_uu = nc.declare_dram_parameter("uu", [S, 128, NCH, 5], BF16, isOutput=False)
    p_w1 = nc.declare_dram_parameter("w1e", [3, HE], BF16, isOutput=False)
    p_w2 = nc.declare_dram_parameter("w2e", [HE, HE], BF16, isOutput=False)
    p_w3 = nc.declare_dram_parameter("w3e", [HE, 2 * K], BF16, isOutput=False)
    p_eps = nc.declare_dram_parameter("epsr", [128, S, 2, D], F32, isOutput=False)
    p_sel = nc.declare_dram_parameter("sel", [128, 5], BF16, isOutput=False)
    # out laid out exactly as the device writes it: [s, chunk-pair, p, j, k]
    p_out = nc.declare_dram_parameter("out", [S, NCH // 4, 128, 4, K], BF16, isOutput=True)
    p_sync = nc.declare_dram_parameter("sync", [1, 8], F32, isOutput=True)
    p_kl = nc.declare_dram_parameter("kl", [2, 128, 2 * S], F32, isOutput=True)

    with tile.TileContext(nc) as tc:
        with (
            tc.tile_pool(name="const", bufs=1) as cst,
            tc.tile_pool(name="io", bufs=4) as iop,
            tc.tile_pool(name="hsb", bufs=4) as hsb,
            tc.tile_pool(name="ppsb", bufs=4) as ppsb,
            tc.tile_pool(name="osb", bufs=4) as osb,
            tc.tile_pool(name="sm", bufs=2) as sm,
            tc.tile_pool(name="pbig", bufs=2, space="PSUM") as pbig,
            tc.tile_pool(name="paux", bufs=2, space="PSUM") as paux,
            tc.tile_pool(name="pred", bufs=2, space="PSUM") as pred,
            tc.tile_pool(name="dram", bufs=2, space="DRAM") as dramp,
        ):
            # ------------------------------------------------ prologue
            # Touch the ACT table set immediately: the (one) table load then
            # runs during the DMA prologue instead of stalling B0's first
            # relu, which would gap the PE stream and re-throttle HAM.
            act_warm = sm.tile([1, 8], F32, name="act_warm")
            nc.vector.memset(act_warm[:], 1.0)
            nc.scalar.activation(act_warm[:], act_warm[:], AF.Exp)
            ut0 = iop.tile([3, NS], BF16, name="ut_sb")
            uu0 = iop.tile([128, NCH, 5], BF16, name="uu_sb")
            nc.scalar.dma_start(ut0[:], p_ut[0])
            nc.scalar.dma_start(uu0[:], p_uu[0])
            w1t = cst.tile([3, HE], BF16)
            w2t = cst.tile([HE, HE], BF16)
            w3t = cst.tile([HE, 2 * K], BF16)
            epsb = cst.tile([128, S, 2, D], F32)
            ident = cst.tile([128, 128], F32)
            dets = cst.tile([128, 2, S], F32)
            kvs = cst.tile([128, 2, S], F32)
            nc.scalar.dma_start(w1t[:], p_w1[:])
            nc.scalar.dma_start(w2t[:], p_w2[:])
            nc.scalar.dma_start(w3t[:], p_w3[:])
            nc.scalar.dma_start(epsb[:], p_eps[:])
            selt = cst.tile([128, 5], BF16)
            nc.scalar.dma_start(selt[:], p_sel[:])
            make_identity(nc, ident[:])
            # Dense matmul burst at t~0: drives the PE HAM activity window
            # busy so the real matmuls run at 2.4GHz instead of 1.2.
            warm_l = cst.tile([128, 128], BF16, name="warm_l")
            warm_r = cst.tile([128, 512], BF16, name="warm_r")
            nc.vector.memset(warm_l[:], 0.0)
            nc.vector.memset(warm_r[:], 0.0)
            wps = pred.tile([128, 512], F32, name="warmps", tag="redsm")
            for i in range(12):
                nc.tensor.matmul(
                    wps[:], warm_l[:], warm_r[:], start=(i == 0), stop=(i == 11)
                )
            wsb2 = sm.tile([1, 8], F32, name="wsb2")
            nc.vector.tensor_copy(wsb2[:], wps[0:1, 0:8])
            nc.gpsimd.dma_start(p_sync[:], wsb2[:])
            ut_t: list = [None] * S
            red_t: list = [None] * S
            ar_t: list = [None] * S

            def phase_A(s):
                if s == 0:
                    ut_t[0] = (ut0, uu0)
                    return
                ut = iop.tile([3, NS], BF16, name="ut_sb")
                uu = iop.tile([128, NCH, 5], BF16, name="uu_sb")
                nc.scalar.dma_start(ut[:], p_ut[s])
                nc.scalar.dma_start(uu[:], p_uu[s])
                ut_t[s] = (ut, uu)

            def phase_B(s):
                ut, uu = ut_t[s]
                red = pred.tile([128, 512], F32, name="red", tag="redsm")
                red_t[s] = red
                h1p = [None] * NG
                h2p = [None] * NG
                h1s = [None] * NG
                h2s = [None] * NG
                for it in range(NG + 2):
                    g0, g1, g2 = it, it - 1, it - 2
                    if 0 <= g1 < NG:
                        # L2 before L1 so the pmlp slot of h1(g1) is provably dead
                        h2p[g1] = pbig.tile([HE, 512], F32, name="hpsum", tag="big")
                        nc.tensor.matmul(
                            h2p[g1][:],
                            w2t[:],
                            h1s[g1][:],
                            start=True,
                            stop=True,
                        )
                    if g0 < NG:
                        h1p[g0] = pbig.tile([HE, 512], F32, name="hpsum", tag="big")
                        nc.tensor.matmul(
                            h1p[g0][:],
                            w1t[:],
                            ut[:, g0 * 512 : (g0 + 1) * 512],
                            start=True,
                            stop=True,
                        )
                    if 0 <= g1 < NG:
                        h2s[g1] = hsb.tile([HE, 512], BF16, name="h2s")
                        nc.vector.tensor_relu(h2s[g1][:], h2p[g1][:])
                    if g0 < NG:
                        h1s[g0] = hsb.tile([HE, 512], BF16, name="h1s")
                        nc.scalar.activation(h1s[g0][:], h1p[g0][:], AF.Relu)
                    if 0 <= g2 < NG:
                        for p in range(2):
                            auxP = paux.tile([128, 2, 512], F32, name="auxP")
                            for i in range(2):
                                j = p * 2 + i
                                nc.tensor.matmul(
                                    auxP[:, i, :],
                                    h2s[g2][:, j * 128 : (j + 1) * 128],
                                    w3t[:],
                                    start=True,
                                    stop=True,
                                )
                            ppt = ppsb.tile([128, 2, 512], BF16, name="ppt")
                            nc.scalar.activation(
                                ppt[:, :, 0:K], auxP[:, :, 0:K], AF.Exp
                            )
                            nc.vector.tensor_mul(
                                ppt[:, :, K : 2 * K],
                                ppt[:, :, 0:K],
                                auxP[:, :, K : 2 * K],
                            )
                            for i in range(2):
                                c = g2 * 4 + p * 2 + i
                                strip = 32 * (c % 4)
                                nc.tensor.matmul(
                                    red[strip : strip + 5, :],
                                    uu[:, c, :],
                                    ppt[:, i, :],
                                    start=(c < 4),
                                    stop=(c >= NCH - 4),
                                    tile_position=(0, strip),
                                )

            def phase_C(s):
                red = red_t[s]
                redsb = sm.tile([128, 2 * K], BF16, name="redsb")
                nc.vector.tensor_copy(redsb[:], red[:])
                rsum = pred.tile([5, 2 * K], F32, name="rsum", tag="redsm")
                nc.tensor.matmul(rsum[:], selt[:], redsb[:], start=True, stop=True)
                rsb = sm.tile([5, 2 * K], F32, name="rsb")
                nc.vector.tensor_copy(rsb[:], rsum[:])
                cc_in = dramp.tile([6, K], F32, name="cc_in")
                cc_out = dramp.tile([6, K], F32, name="cc_out", addr_space="Shared")
                nc.gpsimd.dma_start(cc_in[0:3, :], rsb[0:3, 0:K])
                nc.gpsimd.dma_start(cc_in[3:5, :], rsb[3:5, K : 2 * K])
                # Serialize collectives: row 5 is sync filler copied from the
                # previous sample's collective output, so AllReduce(s) cannot
                # start while AllReduce(s-1) is still in flight.
                if s > 0:
                    nc.gpsimd.dma_start(cc_in[5:6, :], ar_t[s - 1][5:6, :])
                else:
                    nc.gpsimd.dma_start(cc_in[5:6, :], rsb[0:1, 0:K])
                nc.gpsimd.collective_compute(
                    "AllReduce",
                    ALU.add,
                    replica_groups=[list(range(NCORES))],
                    ins=[cc_in[:]],
                    outs=[cc_out[:]],
                )
                ar_t[s] = cc_out

            def phase_D(s):
                """Replicated per-(s,k) 2x2 solves; produces wT [2,256] + kl."""
                ar = sm.tile([5, K], F32, name="ar_sb")
                nc.gpsimd.dma_start(ar[:], ar_t[s][0:5, :])
                Tp = pred.tile([128, 2, 5], F32, name="Tp", tag="redsm")
                for h in range(2):
                    nc.tensor.transpose(
                        Tp[:, h, :], ar[:, h * 128 : (h + 1) * 128], ident[0:5, 0:5]
                    )
                ts = sm.tile([128, 2, 5], F32, name="ts")
                nc.vector.tensor_copy(ts[:], Tp[:])
                a = ts[:, :, 0]
                b = ts[:, :, 1]
                c_ = ts[:, :, 2]
                v0 = ts[:, :, 3]
                v1 = ts[:, :, 4]

                def tmp(nm):
                    return sm.tile([128, 2], F32, name=nm)

                ap1, c1 = tmp("ap1"), tmp("c1")
                nc.vector.tensor_scalar_add(ap1[:], a, 1.0)
                nc.vector.tensor_scalar_add(c1[:], c_, 1.0)
                det, t1, t2 = tmp("det"), tmp("t1"), tmp("t2")
                nc.vector.tensor_mul(t1[:], ap1[:], c1[:])
                nc.vector.tensor_mul(t2[:], b, b)
                nc.vector.tensor_sub(det[:], t1[:], t2[:])
                idet = tmp("idet")
                nc.vector.reciprocal(idet[:], det[:])
                cov00, covp, cov11 = tmp("cov00"), tmp("covp"), tmp("cov11")
                nc.vector.tensor_mul(cov00[:], c1[:], idet[:])
                nc.vector.tensor_mul(covp[:], b, idet[:])  # = -cov01
                nc.vector.tensor_mul(cov11[:], ap1[:], idet[:])
                m0, m1, t3, t4 = tmp("m0"), tmp("m1"), tmp("t3"), tmp("t4")
                nc.vector.tensor_mul(t3[:], cov00[:], v0)
                nc.vector.tensor_mul(t4[:], covp[:], v1)
                nc.vector.tensor_sub(m0[:], t3[:], t4[:])
                nc.vector.tensor_mul(t3[:], covp[:], v0)
                nc.vector.tensor_mul(t4[:], cov11[:], v1)
                nc.vector.tensor_sub(m1[:], t4[:], t3[:])
                # l00 = sqrt(cov00) via exp(0.5 ln x): single pinned ACT set
                l00, l11, pl = tmp("l00"), tmp("l11"), tmp("pl")
                nc.scalar.activation(l00[:], cov00[:], AF.Ln)
                nc.scalar.activation(l00[:], l00[:], AF.Exp, scale=0.5)
                nc.vector.reciprocal(t3[:], l00[:])
                nc.vector.tensor_mul(pl[:], covp[:], t3[:])  # -l10
                nc.vector.tensor_mul(t3[:], pl[:], pl[:])
                nc.vector.tensor_sub(t4[:], cov11[:], t3[:])
                nc.scalar.activation(l11[:], t4[:], AF.Ln)
                nc.scalar.activation(l11[:], l11[:], AF.Exp, scale=0.5)
                e0 = epsb[:, s, :, 0]
                e1 = epsb[:, s, :, 1]
                wsb = sm.tile([128, 2, D], F32, name="wsb")
                nc.vector.tensor_mul(t3[:], l00[:], e0)
                nc.vector.tensor_add(wsb[:, :, 0], m0[:], t3[:])
                nc.vector.tensor_mul(t3[:], pl[:], e0)
                nc.vector.tensor_mul(t4[:], l11[:], e1)
                nc.vector.tensor_sub(t1[:], m1[:], t3[:])
                nc.vector.tensor_add(wsb[:, :, 1], t1[:], t4[:])
                # kl partials: kv = tr + mahal per (s,k); det stored raw.
                # Host finishes kl = 0.5*(sum kv + sum ln det) - K.
                nc.vector.tensor_copy(dets[:, :, s], det[:])
                kv = kvs[:, :, s]
                nc.vector.tensor_add(kv, cov00[:], cov11[:])
                nc.vector.tensor_mul(t3[:], m0[:], m0[:])
                nc.vector.tensor_add(kv, kv, t3[:])
                nc.vector.tensor_mul(t3[:], m1[:], m1[:])
                nc.vector.tensor_add(kv, kv, t3[:])
                return wsb

            def phase_Df(s, wsb):
                wT = sm.tile([2, 2 * 128], BF16, name="wT")
                for h in range(2):
                    wtp = pred.tile([2, 128], F32, name="wtp", tag="redsm")
                    nc.tensor.transpose(wtp[:], wsb[:, h, :], ident[:])
                    nc.vector.tensor_copy(wT[:, h * 128 : (h + 1) * 128], wtp[:])
                return wT

            def phase_E(s, wT):
                # All E phases run after the last B phase, so the aux pool is
                # idle; borrow its 2-bank slots for 4-chunk quads (half the
                # relu instruction count). Relu engine alternates per sample
                # so the next sample's solve chain never queues behind these
                # relus in the DVE FIFO.
                ut, _ = ut_t[s]
                for cq in range(NCH // 4):
                    po = paux.tile([128, 4, K], F32, name="auxP", tag="auxP")
                    for j in range(4):
                        c = cq * 4 + j
                        nc.tensor.matmul(
                            po[:, j, :],
                            ut[0:2, c * 128 : (c + 1) * 128],
                            wT[:],
                            start=True,
                            stop=True,
                        )
                    ost = osb.tile([128, 4, K], BF16, name="ost")
                    if s % 2 == 0:
                        nc.scalar.activation(ost[:], po[:], AF.Relu)
                    else:
                        nc.vector.tensor_relu(ost[:], po[:])
                    nc.sync.dma_start(p_out[s, cq], ost[:])

            # schedule: A0 B0 C0 | A1 B1 C1 | A2 B2 C2 | Ds0 | A3 B3 C3 |
            #           Df0 Ds1 E0 | Df1 Ds2 E1 | Df2 Ds3 E2 | Df3 E3
            # B(s+2) keeps the PE dense across sample-s collective latency.
            wsbs = [None] * S
            wTs = [None] * S
            for s in range(3):
                phase_A(s)
                phase_B(s)
                phase_C(s)
            wsbs[0] = phase_D(0)
            phase_A(3)
            phase_B(3)
            phase_C(3)
            for s in range(S):
                wTs[s] = phase_Df(s, wsbs[s])
                if s + 1 < S:
                    wsbs[s + 1] = phase_D(s + 1)
                phase_E(s, wTs[s])
            nc.scalar.dma_start(p_kl[0], kvs[:])
            nc.scalar.dma_start(p_kl[1], dets[:])

    nc.compile()
    _CACHE["nc"] = nc
    return nc


def _prep_inputs(U, eps, W1, b1, W2, b2, W3, b3):
    f = np.float32
    U = np.asarray(U, f)
    eps = np.asarray(eps, f)
    w1e = np.zeros((3, HE), f)
    w1e[0:2, 0:H] = W1
    w1e[2, 0:H] = b1
    w1e[2, H] = 1.0
    w2e = np.zeros((HE, HE), f)
    w2e[0:H, 0:H] = W2
    w2e[H, 0:H] = b2
    w2e[H, H] = 1.0
    w3e = np.zeros((HE, 2 * K), f)
    w3e[0:H, :] = W3
    w3e[H, :] = b3
    epsr = np.ascontiguousarray(eps.reshape(S, 2, 128, D).transpose(2, 0, 1, 3))
    sel = np.zeros((128, 5), f)
    for i in range(4):
        for q in range(5):
            sel[32 * i + q, q] = 1.0
    w1e, w2e, w3e = (w.astype(BF16NP) for w in (w1e, w2e, w3e))
    in_maps = []
    for c in range(NCORES):
        Us = U[:, c * NS : (c + 1) * NS, :]  # [S, NS, 2]
        ut = np.concatenate(
            [Us.transpose(0, 2, 1), np.ones((S, 1, NS), f)], axis=1
        )  # [S, 3, NS]
        u0, u1 = Us[..., 0], Us[..., 1]
        uu5 = np.stack([u0 * u0, u0 * u1, u1 * u1, u0, u1], axis=-1)  # [S, NS, 5]
        uu = np.ascontiguousarray(
            uu5.reshape(S, NCH, 128, 5).transpose(0, 2, 1, 3)
        )  # [S, 128, NCH, 5]
        in_maps.append(
            {
                "ut": np.ascontiguousarray(ut).astype(BF16NP),
                "uu": uu.astype(BF16NP),
                "w1e": w1e,
                "w2e": w2e,
                "w3e": w3e,
                "epsr": epsr,
                "sel": sel.astype(BF16NP),
            }
        )
    return in_maps


def run(U, eps, W1, b1, W2, b2, W3, b3, trace=False):
    _install_ntff_hook()
    nc = _build()
    in_maps = _prep_inputs(U, eps, W1, b1, W2, b2, W3, b3)
    res = run_bass_kernel_spmd(nc, in_maps, list(range(NCORES)), trace=trace)
    outs = []
    for c in range(NCORES):
        o = res.results[c]["out"]  # [S, NCH//4, 128, 4, K] bf16
        outs.append(
            o.transpose(0, 1, 3, 2, 4).reshape(S, NS, K).astype(np.float32)
        )
    out_U = np.concatenate(outs, axis=1)
    # kl finalize: device ships kv = tr+mahal and det per (s,k); combine here
    klraw = res.results[0]["kl"].reshape(2, 128, 2, S).astype(np.float64)
    kv, det = klraw[0], klraw[1]
    kl = (0.5 * (kv.sum(axis=(0, 1)) + np.log(det).sum(axis=(0, 1))) - K).astype(
        np.float32
    )
    return (out_U, kl), res


def kernel(U, eps, W1, b1, W2, b2, W3, b3):
    (out_U, kl), _ = run(U, eps, W1, b1, W2, b2, W3, b3, trace=False)
    return out_U, kl


# revision 41
# speedup vs baseline: 1.2125x; 1.1849x over previous
"""Trainium2 Bass kernel for the AmortLayer problem.

Math (per sample s of S=4, over N=32768 pseudo-points, K=256 output units,
D=2 input dim, H=100 hidden):
  aux = MLP(U)                      [S,N,2K]  (3-layer, relu)
  prec = exp(aux[..K]); mu = aux[K..]
  UTLU = einsum('snk,snd,sne->skde', prec, U, U)
  UTLv = einsum('snk,snk,snd->skd', prec, mu, U)
  2x2 solve per (s,k): q_cov = (UTLU + I)^-1, q_mu = q_cov UTLv,
  w = q_mu + chol(q_cov) eps, kl per s
  out = relu(U @ w^T)               [S,N,K]

Distribution: data-parallel over N (8 shards of 4096). Each core runs the
MLP + reduction einsums on its shard, a 5KB-per-sample AllReduce combines
the UTLU/UTLv partial sums, the tiny per-(s,k) 2x2 solves are replicated,
and each core emits its N-shard of the output (bf16, upcast on host).

Key device-side choices (arrived at by perfetto-trace iteration):
- All matmul operands bf16 (fp32/f32r stream at 4x/2x cycles per row on
  the PE; bf16 is 1x). PSUM accumulation stays fp32; rel err ~4e-3.
- Biases ride the matmuls via a ones-row/ones-column extension of the
  weight matrices (no separate bias adds), so the MLP is 3 matmuls.
- The n-chunk reduction matmuls are column-packed 4-way with
  tile_position so consecutive chunks run concurrently in disjoint PE
  column groups; a tiny selector matmul sums the four partition strips.
- One pinned ACT table set (natural_log_exp_and_others) for exp/relu/ln:
  the default greedy set selection reloads tables (~2.7us) every time
  the function mix alternates.
- exp/precmu work on paired 2-bank PSUM tiles ([128,2,512]) to halve the
  ACT/DVE instruction count (the 352ns ACT fixed cost dominates at
  [128,256] granularity).
- Engine balance: relu(h1)+exp on ACT, relu(h2)+precmu on DVE. Phase-E
  runs on 4-chunk quad tiles borrowed from the (tail-idle) aux pool,
  with the relu engine alternating per sample so the next sample's
  solve chain never queues behind relus in an engine FIFO.
- Collectives are chained through a sync row in the payload: two
  in-flight AllReduces were the prime suspect for a rare data race (and
  reliably crashed the device in separate experiments).
- DMA rings: bulk output on the sync HWDGE ring; input loads on the
  scalar HWDGE ring; collective bounce + readback on the gpsimd ring.
  (A DMA that waits on a collective blocks its whole FIFO ring, so the
  rings must be separated by latency class.)
- A dense 12-matmul warmup burst at t~0 lifts the PE HAM clock gate to
  2.4GHz before the real work starts, and a dummy activation at t~0
  pulls the one ACT table load into the DMA prologue; the per-(s,k)
  solve is split so the PE-side transposes never wait on the DVE
  solve chain. The strip-sum selector matmul is bf16 (1 cyc/row) to
  keep the B-phase -> AllReduce trigger path short.
- kl is reduced to per-(s,k) partials (tr+mahal, det) shipped as a 8KB
  side output; the host finishes 0.5*(sum kv + sum ln det) - K.

Measured on 8 axon-tunneled trn2 NeuronCores: ~186-230us HW exec
(run-to-run variance is environmental; engine-busy analysis puts the
kernel near the ACT/DVE elementwise floor for this dataflow).
"""

import sys
import types

import ml_dtypes
import numpy as np

BF16NP = ml_dtypes.bfloat16

if "concourse" not in sys.modules:
    for _p in ("/root/.axon_site/_ro/trn_rl_repo", "/opt/trn_rl_repo"):
        if _p not in sys.path:
            sys.path.append(_p)

import bass_rust as _bass_rust
import concourse.bass as bass
import concourse.mybir as mybir
import concourse.tile as tile
from concourse import bacc
from concourse.bass_utils import run_bass_kernel_spmd
from concourse.hw_specs import get_activation_tables
from concourse.masks import make_identity

ACT_SET = "natural_log_exp_and_others"  # exp+ln+relu+copy+square in one set


class _OneActSetBacc(bacc.Bacc):
    """Pin every activation to a single ACT table set.

    The default set-selection is greedy (exp -> exp_and_others,
    ln -> natural_log_*), which re-loads tables ~2.7us each time the
    function mix alternates. Everything this kernel uses lives in
    natural_log_exp_and_others, so empty out the other sets (keeping list
    positions, which are the act_func_set_id walrus expects) and the
    pass emits exactly one load.
    """

    def insert_act_table_loads(self):
        has_activation = any(
            isinstance(i, mybir.InstActivation)
            for b in self.main_func.blocks
            for i in b.instructions
        )
        if not has_activation:
            return
        tables = [
            (name, fns if name == ACT_SET else set())
            for name, fns in get_activation_tables(self.m.arch).items()
        ]
        _bass_rust.insert_act_table_loads(self, tables)

# ---------------------------------------------------------------- constants
S = 4
N = 32768
K = 256
D = 2
H = 100
HE = H + 1  # +1 ones-row carrying biases through the matmuls
NCORES = 8
NS = N // NCORES  # 4096 points per core
NG = NS // 512  # 8 groups of 512 points (L1/L2 granularity)
NCH = NS // 128  # 32 chunks of 128 points (L3/reduction granularity)
F32 = mybir.dt.float32
F32R = mybir.dt.float32r
BF16 = mybir.dt.bfloat16
AF = mybir.ActivationFunctionType
ALU = mybir.AluOpType

_CACHE: dict = {}


def _install_ntff_hook():
    """run_bass_kernel_spmd(trace=True) under axon needs antenv.axon_hooks."""
    if "antenv.axon_hooks" in sys.modules:
        return
    hooks = types.ModuleType("antenv.axon_hooks")
    hooks._HOOK = None

    def _get():
        if hooks._HOOK is None:
            try:
                if "/root/.axon_site" not in sys.path:
                    sys.path.append("/root/.axon_site")
                from trn_agent_boot.trn_boot import _ntff_profile_via_ctypes

                hooks._HOOK = _ntff_profile_via_ctypes("/opt/axon/libaxon_pjrt.so")
            except Exception:
                hooks._HOOK = None
        return hooks._HOOK

    hooks.get_axon_ntff_profile_hook = _get
    hooks.set_axon_ntff_profile_hook = lambda h: setattr(hooks, "_HOOK", h)
    sys.modules["antenv.axon_hooks"] = hooks


def _build():
    """Build + compile the 8-core SPMD Bass graph (cached)."""
    if "nc" in _CACHE:
        return _CACHE["nc"]

    nc = _OneActSetBacc(None)
    p_ut = nc.declare_dram_parameter("ut", [S, 3, NS], BF16, isOutput=False)
    p_uu = nc.declare_dram_parameter("uu", [S, 128, NCH, 5], BF16, isOutput=False)
    p_w1 = nc.declare_dram_parameter("w1e", [3, HE], BF16, isOutput=False)
    p_w2 = nc.declare_dram_parameter("w2e", [HE, HE], BF16, isOutput=False)
    p_w3 = nc.declare_dram_parameter("w3e", [HE, 2 * K], BF16, isOutput=False)
    p_eps = nc.declare_dram_parameter("epsr", [128, S, 2, D], F32, isOutput=False)
    p_sel = nc.declare_dram_parameter("sel", [128, 5], BF16, isOutput=False)
    # out laid out exactly as the device writes it: [s, chunk-pair, p, j, k]
    p_out = nc.declare_dram_parameter("out", [S, NCH // 4, 128, 4, K], BF16, isOutput=True)
    p_sync = nc.declare_dram_parameter("sync", [1, 8], F32, isOutput=True)
    p_kl = nc.declare_dram_parameter("kl", [2, 128, 2 * S], F32, isOutput=True)

    with tile.TileContext(nc) as tc:
        with (
            tc.tile_pool(name="const", bufs=1) as cst,
            tc.tile_pool(name="io", bufs=4) as iop,
            tc.tile_pool(name="hsb", bufs=4) as hsb,
            tc.tile_pool(name="ppsb", bufs=4) as ppsb,
            tc.tile_pool(name="osb", bufs=4) as osb,
            tc.tile_pool(name="sm", bufs=2) as sm,
            tc.tile_pool(name="pbig", bufs=2, space="PSUM") as pbig,
            tc.tile_pool(name="paux", bufs=2, space="PSUM") as paux,
            tc.tile_pool(name="pred", bufs=2, space="PSUM") as pred,
            tc.tile_pool(name="dram", bufs=2, space="DRAM") as dramp,
        ):
            # ------------------------------------------------ prologue
            # Touch the ACT table set immediately: the (one) table load then
            # runs during the DMA prologue instead of stalling B0's first
            # relu, which would gap the PE stream and re-throttle HAM.
            act_warm = sm.tile([1, 8], F32, name="act_warm")
            nc.vector.memset(act_warm[:], 1.0)
            nc.scalar.activation(act_warm[:], act_warm[:], AF.Exp)
            ut0 = iop.tile([3, NS], BF16, name="ut_sb")
            uu0 = iop.tile([128, NCH, 5], BF16, name="uu_sb")
            nc.scalar.dma_start(ut0[:], p_ut[0])
            nc.scalar.dma_start(uu0[:], p_uu[0])
            w1t = cst.tile([3, HE], BF16)
            w2t = cst.tile([HE, HE], BF16)
            w3t = cst.tile([HE, 2 * K], BF16)
            epsb = cst.tile([128, S, 2, D], F32)
            ident = cst.tile([128, 128], F32)
            dets = cst.tile([128, 2, S], F32)
            kvs = cst.tile([128, 2, S], F32)
            nc.scalar.dma_start(w1t[:], p_w1[:])
            nc.scalar.dma_start(w2t[:], p_w2[:])
            nc.scalar.dma_start(w3t[:], p_w3[:])
            nc.scalar.dma_start(epsb[:], p_eps[:])
            selt = cst.tile([128, 5], BF16)
            nc.scalar.dma_start(selt[:], p_sel[:])
            make_identity(nc, ident[:])
            # Dense matmul burst at t~0: drives the PE HAM activity window
            # busy so the real matmuls run at 2.4GHz instead of 1.2.
            warm_l = cst.tile([128, 128], BF16, name="warm_l")
            warm_r = cst.tile([128, 512], BF16, name="warm_r")
            nc.vector.memset(warm_l[:], 0.0)
            nc.vector.memset(warm_r[:], 0.0)
            wps = pred.tile([128, 512], F32, name="warmps", tag="redsm")
            for i in range(12):
                nc.tensor.matmul(
                    wps[:], warm_l[:], warm_r[:], start=(i == 0), stop=(i == 11)
                )
            wsb2 = sm.tile([1, 8], F32, name="wsb2")
            nc.vector.tensor_copy(wsb2[:], wps[0:1, 0:8])
            nc.gpsimd.dma_start(p_sync[:], wsb2[:])
            ut_t: list = [None] * S
            red_t: list = [None] * S
            ar_t: list = [None] * S

            def phase_A(s):
                if s == 0:
                    ut_t[0] = (ut0, uu0)
                    return
                ut = iop.tile([3, NS], BF16, name="ut_sb")
                uu = iop.tile([128, NCH, 5], BF16, name="uu_sb")
                nc.scalar.dma_start(ut[:], p_ut[s])
                nc.scalar.dma_start(uu[:], p_uu[s])
                ut_t[s] = (ut, uu)

            def phase_B(s):
                ut, uu = ut_t[s]
                red = pred.tile([128, 512], F32, name="red", tag="redsm")
                red_t[s] = red
                h1p = [None] * NG
                h2p = [None] * NG
                h1s = [None] * NG
                h2s = [None] * NG
                for it in range(NG + 2):
                    g0, g1, g2 = it, it - 1, it - 2
                    if 0 <= g1 < NG:
                        # L2 before L1 so the pmlp slot of h1(g1) is provably dead
                        h2p[g1] = pbig.tile([HE, 512], F32, name="hpsum", tag="big")
                        nc.tensor.matmul(
                            h2p[g1][:],
                            w2t[:],
                            h1s[g1][:],
                            start=True,
                            stop=True,
                        )
                    if g0 < NG:
                        h1p[g0] = pbig.tile([HE, 512], F32, name="hpsum", tag="big")
                        nc.tensor.matmul(
                            h1p[g0][:],
                            w1t[:],
                            ut[:, g0 * 512 : (g0 + 1) * 512],
                            start=True,
                            stop=True,
                        )
                    if 0 <= g1 < NG:
                        h2s[g1] = hsb.tile([HE, 512], BF16, name="h2s")
                        nc.vector.tensor_relu(h2s[g1][:], h2p[g1][:])
                    if g0 < NG:
                        h1s[g0] = hsb.tile([HE, 512], BF16, name="h1s")
                        nc.scalar.activation(h1s[g0][:], h1p[g0][:], AF.Relu)
                    if 0 <= g2 < NG:
                        pps = []
                        for p in range(2):
                            auxP = paux.tile([128, 2, 512], F32, name="auxP")
                            for i in range(2):
                                j = p * 2 + i
                                nc.tensor.matmul(
                                    auxP[:, i, :],
                                    h2s[g2][:, j * 128 : (j + 1) * 128],
                                    w3t[:],
                                    start=True,
                                    stop=True,
                                )
                            ppt = ppsb.tile([128, 2, 512], BF16, name="ppt")
                            nc.scalar.activation(
                                ppt[:, :, 0:K], auxP[:, :, 0:K], AF.Exp
                            )
                            nc.vector.tensor_mul(
                                ppt[:, :, K : 2 * K],
                                ppt[:, :, 0:K],
                                auxP[:, :, K : 2 * K],
                            )
                            pps.append(ppt)
                        # all four reduction matmuls adjacent in the PE FIFO
                        # so the 4-way column-group packing actually overlaps
                        for p in range(2):
                            for i in range(2):
                                c = g2 * 4 + p * 2 + i
                                strip = 32 * (c % 4)
                                nc.tensor.matmul(
                                    red[strip : strip + 5, :],
                                    uu[:, c, :],
                                    pps[p][:, i, :],
                                    start=(c < 4),
                                    stop=(c >= NCH - 4),
                                    tile_position=(0, strip),
                                )

            def phase_C(s):
                red = red_t[s]
                redsb = sm.tile([128, 2 * K], BF16, name="redsb")
                nc.vector.tensor_copy(redsb[:], red[:])
                rsum = pred.tile([5, 2 * K], F32, name="rsum", tag="redsm")
                nc.tensor.matmul(rsum[:], selt[:], redsb[:], start=True, stop=True)
                rsb = sm.tile([5, 2 * K], F32, name="rsb")
                nc.vector.tensor_copy(rsb[:], rsum[:])
                cc_in = dramp.tile([6, K], F32, name="cc_in")
                cc_out = dramp.tile([6, K], F32, name="cc_out", addr_space="Shared")
                nc.gpsimd.dma_start(cc_in[0:3, :], rsb[0:3, 0:K])
                nc.gpsimd.dma_start(cc_in[3:5, :], rsb[3:5, K : 2 * K])
                # Serialize collectives: row 5 is sync filler copied from the
                # previous sample's collective output, so AllReduce(s) cannot
                # start while AllReduce(s-1) is still in flight.
                if s > 0:
                    nc.gpsimd.dma_start(cc_in[5:6, :], ar_t[s - 1][5:6, :])
                else:
                    nc.gpsimd.dma_start(cc_in[5:6, :], rsb[0:1, 0:K])
                nc.gpsimd.collective_compute(
                    "AllReduce",
                    ALU.add,
                    replica_groups=[list(range(NCORES))],
                    ins=[cc_in[:]],
                    outs=[cc_out[:]],
                )
                ar_t[s] = cc_out

            def phase_D(s):
                """Replicated per-(s,k) 2x2 solves; produces wT [2,256] + kl."""
                ar = sm.tile([5, K], F32, name="ar_sb")
                nc.gpsimd.dma_start(ar[:], ar_t[s][0:5, :])
                Tp = pred.tile([128, 2, 5], F32, name="Tp", tag="redsm")
                for h in range(2):
                    nc.tensor.transpose(
                        Tp[:, h, :], ar[:, h * 128 : (h + 1) * 128], ident[0:5, 0:5]
                    )
                ts = sm.tile([128, 2, 5], F32, name="ts")
                nc.vector.tensor_copy(ts[:], Tp[:])
                a = ts[:, :, 0]
                b = ts[:, :, 1]
                c_ = ts[:, :, 2]
                v0 = ts[:, :, 3]
                v1 = ts[:, :, 4]

                def tmp(nm):
                    return sm.tile([128, 2], F32, name=nm)

                ap1, c1 = tmp("ap1"), tmp("c1")
                nc.vector.tensor_scalar_add(ap1[:], a, 1.0)
                nc.vector.tensor_scalar_add(c1[:], c_, 1.0)
                det, t1, t2 = tmp("det"), tmp("t1"), tmp("t2")
                nc.vector.tensor_mul(t1[:], ap1[:], c1[:])
                nc.vector.tensor_mul(t2[:], b, b)
                nc.vector.tensor_sub(det[:], t1[:], t2[:])
                idet = tmp("idet")
                nc.vector.reciprocal(idet[:], det[:])
                cov00, covp, cov11 = tmp("cov00"), tmp("covp"), tmp("cov11")
                nc.vector.tensor_mul(cov00[:], c1[:], idet[:])
                nc.vector.tensor_mul(covp[:], b, idet[:])  # = -cov01
                nc.vector.tensor_mul(cov11[:], ap1[:], idet[:])
                m0, m1, t3, t4 = tmp("m0"), tmp("m1"), tmp("t3"), tmp("t4")
                nc.vector.tensor_mul(t3[:], cov00[:], v0)
                nc.vector.tensor_mul(t4[:], covp[:], v1)
                nc.vector.tensor_sub(m0[:], t3[:], t4[:])
                nc.vector.tensor_mul(t3[:], covp[:], v0)
                nc.vector.tensor_mul(t4[:], cov11[:], v1)
                nc.vector.tensor_sub(m1[:], t4[:], t3[:])
                # l00 = sqrt(cov00) via exp(0.5 ln x): single pinned ACT set
                l00, l11, pl = tmp("l00"), tmp("l11"), tmp("pl")
                nc.scalar.activation(l00[:], cov00[:], AF.Ln)
                nc.scalar.activation(l00[:], l00[:], AF.Exp, scale=0.5)
                nc.vector.reciprocal(t3[:], l00[:])
                nc.vector.tensor_mul(pl[:], covp[:], t3[:])  # -l10
                nc.vector.tensor_mul(t3[:], pl[:], pl[:])
                nc.vector.tensor_sub(t4[:], cov11[:], t3[:])
                nc.scalar.activation(l11[:], t4[:], AF.Ln)
                nc.scalar.activation(l11[:], l11[:], AF.Exp, scale=0.5)
                e0 = epsb[:, s, :, 0]
                e1 = epsb[:, s, :, 1]
                wsb = sm.tile([128, 2, D], F32, name="wsb")
                nc.vector.tensor_mul(t3[:], l00[:], e0)
                nc.vector.tensor_add(wsb[:, :, 0], m0[:], t3[:])
                nc.vector.tensor_mul(t3[:], pl[:], e0)
                nc.vector.tensor_mul(t4[:], l11[:], e1)
                nc.vector.tensor_sub(t1[:], m1[:], t3[:])
                nc.vector.tensor_add(wsb[:, :, 1], t1[:], t4[:])
                # kl partials: kv = tr + mahal per (s,k); det stored raw.
                # Host finishes kl = 0.5*(sum kv + sum ln det) - K.
                nc.vector.tensor_copy(dets[:, :, s], det[:])
                kv = kvs[:, :, s]
                nc.vector.tensor_add(kv, cov00[:], cov11[:])
                nc.vector.tensor_mul(t3[:], m0[:], m0[:])
                nc.vector.tensor_add(kv, kv, t3[:])
                nc.vector.tensor_mul(t3[:], m1[:], m1[:])
                nc.vector.tensor_add(kv, kv, t3[:])
                return wsb

            def phase_Df(s, wsb):
                wT = sm.tile([2, 2 * 128], BF16, name="wT")
                for h in range(2):
                    wtp = pred.tile([2, 128], F32, name="wtp", tag="redsm")
                    nc.tensor.transpose(wtp[:], wsb[:, h, :], ident[:])
                    nc.vector.tensor_copy(wT[:, h * 128 : (h + 1) * 128], wtp[:])
                return wT

            def phase_E(s, wT):
                # All E phases run after the last B phase, so the aux pool is
                # idle; borrow its 2-bank slots for 4-chunk quads (half the
                # relu instruction count). Relu engine alternates per sample
                # so the next sample's solve chain never queues behind these
                # relus in the DVE FIFO.
                ut, _ = ut_t[s]
                for cq in range(NCH // 4):
                    po = paux.tile([128, 4, K], F32, name="auxP", tag="auxP")
                    for j in range(4):
                        c = cq * 4 + j
                        nc.tensor.matmul(
                            po[:, j, :],
                            ut[0:2, c * 128 : (c + 1) * 128],
                            wT[:],
                            start=True,
                            stop=True,
                        )
                    ost = osb.tile([128, 4, K], BF16, name="ost")
                    if s % 2 == 0:
                        nc.scalar.activation(ost[:], po[:], AF.Relu)
                    else:
                        nc.vector.tensor_relu(ost[:], po[:])
                    nc.sync.dma_start(p_out[s, cq], ost[:])

            # schedule: A0 B0 C0 | A1 B1 C1 | A2 B2 C2 | Ds0 | A3 B3 C3 |
            #           Df0 Ds1 E0 | Df1 Ds2 E1 | Df2 Ds3 E2 | Df3 E3
            # B(s+2) keeps the PE dense across sample-s collective latency.
            wsbs = [None] * S
            wTs = [None] * S
            for s in range(3):
                phase_A(s)
                phase_B(s)
                phase_C(s)
            wsbs[0] = phase_D(0)
            phase_A(3)
            phase_B(3)
            phase_C(3)
            for s in range(S):
                wTs[s] = phase_Df(s, wsbs[s])
                if s + 1 < S:
                    wsbs[s + 1] = phase_D(s + 1)
                phase_E(s, wTs[s])
            nc.scalar.dma_start(p_kl[0], kvs[:])
            nc.scalar.dma_start(p_kl[1], dets[:])

    nc.compile()
    _CACHE["nc"] = nc
    return nc


def _prep_inputs(U, eps, W1, b1, W2, b2, W3, b3):
    f = np.float32
    U = np.asarray(U, f)
    eps = np.asarray(eps, f)
    w1e = np.zeros((3, HE), f)
    w1e[0:2, 0:H] = W1
    w1e[2, 0:H] = b1
    w1e[2, H] = 1.0
    w2e = np.zeros((HE, HE), f)
    w2e[0:H, 0:H] = W2
    w2e[H, 0:H] = b2
    w2e[H, H] = 1.0
    w3e = np.zeros((HE, 2 * K), f)
    w3e[0:H, :] = W3
    w3e[H, :] = b3
    epsr = np.ascontiguousarray(eps.reshape(S, 2, 128, D).transpose(2, 0, 1, 3))
    sel = np.zeros((128, 5), f)
    for i in range(4):
        for q in range(5):
            sel[32 * i + q, q] = 1.0
    w1e, w2e, w3e = (w.astype(BF16NP) for w in (w1e, w2e, w3e))
    in_maps = []
    for c in range(NCORES):
        Us = U[:, c * NS : (c + 1) * NS, :]  # [S, NS, 2]
        ut = np.concatenate(
            [Us.transpose(0, 2, 1), np.ones((S, 1, NS), f)], axis=1
        )  # [S, 3, NS]
        u0, u1 = Us[..., 0], Us[..., 1]
        uu5 = np.stack([u0 * u0, u0 * u1, u1 * u1, u0, u1], axis=-1)  # [S, NS, 5]
        uu = np.ascontiguousarray(
            uu5.reshape(S, NCH, 128, 5).transpose(0, 2, 1, 3)
        )  # [S, 128, NCH, 5]
        in_maps.append(
            {
                "ut": np.ascontiguousarray(ut).astype(BF16NP),
                "uu": uu.astype(BF16NP),
                "w1e": w1e,
                "w2e": w2e,
                "w3e": w3e,
                "epsr": epsr,
                "sel": sel.astype(BF16NP),
            }
        )
    return in_maps


def run(U, eps, W1, b1, W2, b2, W3, b3, trace=False):
    _install_ntff_hook()
    nc = _build()
    in_maps = _prep_inputs(U, eps, W1, b1, W2, b2, W3, b3)
    res = run_bass_kernel_spmd(nc, in_maps, list(range(NCORES)), trace=trace)
    outs = []
    for c in range(NCORES):
        o = res.results[c]["out"]  # [S, NCH//4, 128, 4, K] bf16
        outs.append(
            o.transpose(0, 1, 3, 2, 4).reshape(S, NS, K).astype(np.float32)
        )
    out_U = np.concatenate(outs, axis=1)
    # kl finalize: device ships kv = tr+mahal and det per (s,k); combine here
    klraw = res.results[0]["kl"].reshape(2, 128, 2, S).astype(np.float64)
    kv, det = klraw[0], klraw[1]
    kl = (0.5 * (kv.sum(axis=(0, 1)) + np.log(det).sum(axis=(0, 1))) - K).astype(
        np.float32
    )
    return (out_U, kl), res


def kernel(U, eps, W1, b1, W2, b2, W3, b3):
    (out_U, kl), _ = run(U, eps, W1, b1, W2, b2, W3, b3, trace=False)
    return out_U, kl
